# revision 15
# baseline (speedup 1.0000x reference)
"""Trainium2 Bass kernel for MQA attention with per-expert QKV/out projections.

Problem (hardcoded shapes):
  B=4, T0=1024 (W0=2048 expert), T1=256 (W1=1024 expert), cache 1024, S=2304,
  N=8 query heads, KV=1 (MQA), H=256.  Returns (out0, out1, k, v).

Sharding: 8 cores = (batch b, token-half h): core c -> b=c//2, h=c%2.
Each core projects q/k/v for its 640 tokens (512 from x0, 128 from x1),
all-gathers k/v with its batch peer, runs attention for all 8 heads over
the full 2304-key axis in transposed (logitsT = [S, T]) layout, and applies
the output projections for its tokens.  Matmuls run as float32r.

The attention mask is handled per 128(s) x 320(t) block, classified from the
actual input (union over cores): fully-masked blocks are skipped, fully-open
blocks need no mask op, mixed blocks get an additive mask loaded from DRAM.
"""
import numpy as np
from contextlib import ExitStack

import concourse.bacc as bacc
import concourse.tile as tile
import concourse.mybir as mybir
from concourse.alu_op_type import AluOpType
from concourse.bass_utils import run_bass_kernel_spmd

F32 = mybir.dt.float32
F32R = mybir.dt.float32r
AF = mybir.ActivationFunctionType
PI = float(np.pi)
MAGIC = float(2**23)

B, T0, T1, SCC = 4, 1024, 256, 1024
T = T0 + T1                 # 1280
S = SCC + T                 # 2304
W0, W1 = 2048, 1024
NH, H = 8, 256
SCALE = float(H**-0.5)      # 1/16

N_CORES = 8
T0C, T1C = T0 // 2, T1 // 2  # 512, 128 per core
TC = T0C + T1C               # 640
TCH = 320                    # attention t-chunk (>=256 for fp32r full rate)
N_TCH = TC // TCH            # 2
N_SCH = S // 128             # 18 s-chunks
BIG_NEG = -1.0e30
ZERO, ONES, MIXED = 0, 1, 2

_CACHE = {}


def _range_reduce_sin_cos(nc, pool, rad_ps, sin_dst, cos_dst, width):
    """From radians psum tile [128, width], write sin/cos into dst APs.

    sin arg reduced to [-pi, pi] via r - 2pi*round(r/2pi) (fp32 magic-round);
    cos arg reduced via r + pi/2 - 2pi*round(r/2pi + 0.25).
    """
    a = pool.tile([128, width], F32, tag="rr_a", name="rr_a")
    nc.vector.tensor_scalar(a[:], rad_ps[:], 1.0 / (2 * PI), MAGIC,
                            AluOpType.mult, AluOpType.add)
    b = pool.tile([128, width], F32, tag="rr_b", name="rr_b")
    nc.vector.tensor_scalar(b[:], a[:], MAGIC, -2 * PI,
                            AluOpType.subtract, AluOpType.mult)
    m = pool.tile([128, width], F32, tag="rr_c", name="rr_c")
    nc.vector.tensor_tensor(m[:], rad_ps[:], b[:], AluOpType.add)
    nc.scalar.activation(sin_dst, m[:], AF.Sin)

    a2 = pool.tile([128, width], F32, tag="rr_a", name="rr_a2")
    nc.vector.tensor_scalar(a2[:], rad_ps[:], 1.0 / (2 * PI), 0.25,
                            AluOpType.mult, AluOpType.add)
    b2 = pool.tile([128, width], F32, tag="rr_b", name="rr_b2")
    nc.vector.tensor_scalar(b2[:], a2[:], MAGIC, MAGIC,
                            AluOpType.add, AluOpType.subtract)
    c2 = pool.tile([128, width], F32, tag="rr_c", name="rr_c2")
    nc.vector.tensor_scalar(c2[:], b2[:], -2 * PI, PI / 2,
                            AluOpType.mult, AluOpType.add)
    m2 = pool.tile([128, width], F32, tag="rr_d", name="rr_d2")
    nc.vector.tensor_tensor(m2[:], rad_ps[:], c2[:], AluOpType.add)
    nc.scalar.activation(cos_dst, m2[:], AF.Sin)


def _rope_pair(nc, pool, out0_ap, out1_ap, p0, p1, ss, cc, width, tag):
    """out0 = p0*cc - p1*ss ; out1 = p1*cc + p0*ss (elementwise [128, width])."""
    t1 = pool.tile([128, width], F32, tag="rope_t1", name=f"rt1_{tag}")
    nc.vector.tensor_tensor(t1[:], p0, cc, AluOpType.mult)
    t2 = pool.tile([128, width], F32, tag="rope_t2", name=f"rt2_{tag}")
    nc.vector.tensor_tensor(t2[:], p1, ss, AluOpType.mult)
    nc.vector.tensor_tensor(out0_ap, t1[:], t2[:], AluOpType.subtract)
    t3 = pool.tile([128, width], F32, tag="rope_t1", name=f"rt3_{tag}")
    nc.vector.tensor_tensor(t3[:], p1, cc, AluOpType.mult)
    t4 = pool.tile([128, width], F32, tag="rope_t2", name=f"rt4_{tag}")
    nc.vector.tensor_tensor(t4[:], p0, ss, AluOpType.mult)
    nc.vector.tensor_tensor(out1_ap, t3[:], t4[:], AluOpType.add)


def _build(cls):
    """cls[tcn][sc] in {ZERO, ONES, MIXED} — mask block classification."""
    nc = bacc.Bacc(trn_type="TRN2", debug=False, num_devices=N_CORES)

    def din(name, shape):
        return nc.dram_tensor(name, shape, F32, kind="ExternalInput").ap()

    def dout(name, shape):
        return nc.dram_tensor(name, shape, F32, kind="ExternalOutput").ap()

    x0T_d = din("x0T", [W0, T0C])          # this core's x0 tokens, transposed
    x1T_d = din("x1T", [W1, T1C])
    wq0_d = din("wq0", [NH, W0, H])
    wkv0_d = din("wkv0", [2, W0, H])
    wq1_d = din("wq1", [NH, W1, H])
    wkv1_d = din("wkv1", [2, W1, H])
    wo0_d = din("wo0f", [NH * H, W0])      # flattened (n,h)-major
    wo1_d = din("wo1f", [NH * H, W1])
    ckT_d = din("ckT", [H, SCC])           # cache_k[b] transposed
    cv_d = din("cv", [SCC, H])             # cache_v[b]
    pos_d = din("pos", [1, TC])            # positions of this core's tokens
    invts_d = din("invts", [1, 128])       # 1/timescale
    ident_d = din("ident", [128, 128])     # identity for PE transpose
    maskT_d = din("maskT", [S, TC])        # additive mask, [s, t] layout

    out0_d = dout("out0", [T0C, W0])
    out1_d = dout("out1", [T1C, W1])
    kTn_d = dout("kTn", [H, TC])           # roped new k, [H, t]
    vn_d = dout("vn", [TC, H])             # new v, [t, H]

    with tile.TileContext(nc, num_cores=N_CORES) as tc, ExitStack() as ctx:
        # ---- whole-kernel pools ----
        pers = ctx.enter_context(tc.tile_pool(name="pers", bufs=1))
        qT_pool = ctx.enter_context(tc.tile_pool(name="qT", bufs=NH))
        encT_pool = ctx.enter_context(tc.tile_pool(name="encT", bufs=NH))
        dram = ctx.enter_context(tc.tile_pool(name="dram", bufs=1, space="DRAM"))

        ident = pers.tile([128, 128], F32)
        nc.sync.dma_start(ident[:], ident_d[:])
        ones_f = pers.tile([128, 128], F32)
        nc.vector.memset(ones_f[:], 1.0)
        ones_r = pers.tile([128, 128], F32R)
        nc.vector.tensor_copy(ones_r[:], ones_f[:])
        pos_sb = pers.tile([1, TC], F32)
        nc.sync.dma_start(pos_sb[:], pos_d[:])
        invts_sb = pers.tile([1, 128], F32)
        nc.sync.dma_start(invts_sb[:], invts_d[:])

        # rope tables ([i, t] layout for H-major rope; [t, i] for x1 path)
        sinT = pers.tile([128, TC], F32)
        cosT = pers.tile([128, TC], F32)
        sinTq = pers.tile([128, TC], F32)
        cosTq = pers.tile([128, TC], F32)
        sinF = pers.tile([128, 128], F32)
        cosF = pers.tile([128, 128], F32)
        sinFq = pers.tile([128, 128], F32)
        cosFq = pers.tile([128, 128], F32)
        with tc.tile_pool(name="ps_rad", bufs=2, space="PSUM") as ps_rad, \
             tc.tile_pool(name="rrtmp", bufs=2) as rrtmp:
            for half in range(2):
                sl = slice(half * TCH, (half + 1) * TCH)
                rad = ps_rad.tile([128, TCH], F32, tag="rad", name=f"radT{half}")
                nc.tensor.matmul(rad[:], invts_sb[:], pos_sb[:, sl],
                                 start=True, stop=True)
                _range_reduce_sin_cos(nc, rrtmp, rad, sinT[:, sl],
                                      cosT[:, sl], TCH)
            radF = ps_rad.tile([128, 128], F32, tag="rad", name="radF")
            nc.tensor.matmul(radF[:], pos_sb[:, T0C:TC], invts_sb[:],
                             start=True, stop=True)
            _range_reduce_sin_cos(nc, rrtmp, radF, sinF[:], cosF[:], 128)
        for src, dst in ((sinT, sinTq), (cosT, cosTq), (sinF, sinFq),
                         (cosF, cosFq)):
            nc.vector.tensor_scalar(dst[:], src[:], SCALE, None, AluOpType.mult)

        qts = [qT_pool.tile([128, 2 * TC], F32R, tag="qT", name=f"qT_{n}")
               for n in range(NH)]

        # ---- projections (x0T/x1T + weight streams scoped to this block) ----
        with tc.tile_pool(name="xstage", bufs=1) as xstage, \
             tc.tile_pool(name="kvtmp", bufs=1) as kvtmp, \
             tc.tile_pool(name="tmp", bufs=2) as tmp_pool, \
             tc.tile_pool(name="wbig", bufs=3) as wbig, \
             tc.tile_pool(name="wsmall", bufs=2) as wsmall, \
             tc.tile_pool(name="ps_qp", bufs=3, space="PSUM") as ps_qp, \
             tc.tile_pool(name="ps_vp", bufs=2, space="PSUM") as ps_vp, \
             tc.tile_pool(name="ps_tr", bufs=2, space="PSUM") as ps_tr:

            def load_w16(w_ap, name):
                """Load a [2048, 256] weight as two [128, 8*256] sbuf tiles."""
                src = w_ap.rearrange("(c p) h -> p c h", p=128).bitcast(F32R)
                tiles = []
                for hf in range(2):
                    wt = wbig.tile([128, 8 * H], F32R, tag="wbig",
                                   name=f"{name}_{hf}")
                    nc.sync.dma_start(
                        wt[:].rearrange("p (c h) -> p c h", c=8),
                        src[:, hf * 8:(hf + 1) * 8, :])
                    tiles.append(wt)

                def sl(dc, off=0, width=H):
                    base = (dc % 8) * H + off
                    return tiles[dc // 8][:, base: base + width]
                return sl

            x0T_sb = xstage.tile([128, 16 * T0C], F32R)   # [128, (dc t)]
            nc.sync.dma_start(
                x0T_sb[:].rearrange("p (c t) -> p c t", c=16),
                x0T_d.rearrange("(c p) t -> p c t", p=128).bitcast(F32R))
            x1T_sb = xstage.tile([128, 8 * T1C], F32R)
            nc.sync.dma_start(
                x1T_sb[:].rearrange("p (c t) -> p c t", c=8),
                x1T_d.rearrange("(c p) t -> p c t", p=128).bitcast(F32R))

            ktmp = kvtmp.tile([128, 2 * TC], F32)    # new k, [hc, t] free-major
            vtmp = kvtmp.tile([128, 5 * 256], F32)   # new v, 5 token-chunks

            # x0 -> q (per head, [H,t] layout), via lhsT=wq0 chunks
            for n in range(NH):
                wq = load_w16(wq0_d[n], f"wq0_{n}")
                qps = []
                for hc in range(2):
                    qp = ps_qp.tile([128, T0C], F32, tag="qp",
                                    name=f"qp_{n}_{hc}")
                    for dc in range(16):
                        nc.tensor.matmul(
                            qp[:],
                            wq(dc, hc * 128, 128),
                            x0T_sb[:, dc * T0C: (dc + 1) * T0C],
                            start=(dc == 0), stop=(dc == 15))
                    qps.append(qp)
                _rope_pair(nc, tmp_pool, qts[n][:, 0:T0C],
                           qts[n][:, TC:TC + T0C], qps[0][:], qps[1][:],
                           sinTq[:, 0:T0C], cosTq[:, 0:T0C], T0C, f"q{n}")

            # x0 -> k ([H,t] layout)
            wk = load_w16(wkv0_d[0], "wk0")
            kps = []
            for hc in range(2):
                kp = ps_qp.tile([128, T0C], F32, tag="qp", name=f"kp_{hc}")
                for dc in range(16):
                    nc.tensor.matmul(
                        kp[:],
                        wk(dc, hc * 128, 128),
                        x0T_sb[:, dc * T0C: (dc + 1) * T0C],
                        start=(dc == 0), stop=(dc == 15))
                kps.append(kp)
            _rope_pair(nc, tmp_pool, ktmp[:, 0:T0C], ktmp[:, TC:TC + T0C],
                       kps[0][:], kps[1][:], sinT[:, 0:T0C], cosT[:, 0:T0C],
                       T0C, "k")

            # x0 -> v ([t, H] layout)
            wv = load_w16(wkv0_d[1], "wv0")
            for tcn in range(4):
                vp = ps_vp.tile([128, H], F32, tag="vp", name=f"vp_{tcn}")
                for dc in range(16):
                    nc.tensor.matmul(
                        vp[:],
                        x0T_sb[:, dc * T0C + tcn * 128: dc * T0C + tcn * 128 + 128],
                        wv(dc),
                        start=(dc == 0), stop=(dc == 15))
                nc.vector.tensor_copy(vtmp[:, tcn * H: (tcn + 1) * H], vp[:])

            # x1 -> q/k in [t, H] then PE-transpose into [H, t]
            def x1_proj(w_d, name):
                ww = wsmall.tile([128, 8 * H], F32R, tag="wsmall",
                                 name=f"w_{name}")
                nc.sync.dma_start(
                    ww[:].rearrange("p (c h) -> p c h", c=8),
                    w_d.rearrange("(c p) h -> p c h", p=128).bitcast(F32R))
                pp = ps_vp.tile([128, H], F32, tag="vp", name=f"pp_{name}")
                for dc in range(8):
                    nc.tensor.matmul(
                        pp[:],
                        x1T_sb[:, dc * T1C: (dc + 1) * T1C],
                        ww[:, dc * H: (dc + 1) * H],
                        start=(dc == 0), stop=(dc == 7))
                return pp

            def rope_F(pp, scaled, name):
                ss, cc = (sinFq, cosFq) if scaled else (sinF, cosF)
                ro = tmp_pool.tile([128, H], F32, tag="ropeF", name=f"ro_{name}")
                _rope_pair(nc, tmp_pool, ro[:, 0:128], ro[:, 128:256],
                           pp[:, 0:128], pp[:, 128:256], ss[:], cc[:], 128,
                           name)
                return ro

            def transpose_to(ro, out_tile, base_off, name):
                for hc in range(2):
                    tp = ps_tr.tile([128, 128], F32, tag="tr",
                                    name=f"tp_{name}_{hc}")
                    nc.tensor.transpose(tp[:], ro[:, hc * 128:(hc + 1) * 128],
                                        ident[:])
                    nc.vector.tensor_copy(
                        out_tile[:, base_off + hc * TC:
                                 base_off + hc * TC + T1C], tp[:])

            for n in range(NH):
                pp = x1_proj(wq1_d[n], f"q1_{n}")
                ro = rope_F(pp, True, f"q1_{n}")
                transpose_to(ro, qts[n], T0C, f"q1_{n}")

            pp = x1_proj(wkv1_d[0], "k1")
            ro = rope_F(pp, False, "k1")
            transpose_to(ro, ktmp, T0C, "k1")

            pp = x1_proj(wkv1_d[1], "v1")
            nc.vector.tensor_copy(vtmp[:, 4 * H: 5 * H], pp[:])

            # ---- outputs for new k/v + allgather ----
            for hc in range(2):
                nc.sync.dma_start(kTn_d[hc * 128:(hc + 1) * 128, :],
                                  ktmp[:, hc * TC:(hc + 1) * TC])
            nc.sync.dma_start(vn_d.rearrange("(c p) h -> p c h", p=128),
                              vtmp[:].rearrange("p (c h) -> p c h", c=5))

            KBLOB = 2 * 128 * TC        # 163840
            VBLOB = 5 * 128 * H         # 163840
            bnc_in = dram.tile([KBLOB + VBLOB], F32)
            bnc_out = dram.tile([2, KBLOB + VBLOB], F32)
            nc.sync.dma_start(
                bnc_in[0:KBLOB].rearrange("(hc p t) -> p hc t", hc=2, p=128),
                ktmp[:].rearrange("p (hc t) -> p hc t", hc=2))
            nc.sync.dma_start(
                bnc_in[KBLOB:KBLOB + VBLOB].rearrange("(c p h) -> p c h",
                                                      c=5, p=128),
                vtmp[:].rearrange("p (c h) -> p c h", c=5))
            nc.gpsimd.collective_compute(
                "AllGather", mybir.AluOpType.bypass,
                replica_groups=[[0, 1], [2, 3], [4, 5], [6, 7]],
                ins=[bnc_in[:]], outs=[bnc_out[:]])

        # ---- assemble kT [128, (hc s)] and v [128, (sc h)] ----
        kv_pool = ctx.enter_context(tc.tile_pool(name="kv", bufs=1))
        kT_sb = kv_pool.tile([128, 2 * S], F32R)
        for hc in range(2):
            nc.sync.dma_start(
                kT_sb[:, hc * S: hc * S + SCC],
                ckT_d[hc * 128:(hc + 1) * 128, :].bitcast(F32R))
        v_sb = kv_pool.tile([128, N_SCH * H], F32R)
        nc.sync.dma_start(
            v_sb[:, 0: 8 * H].rearrange("p (c h) -> p c h", c=8),
            cv_d.rearrange("(c p) h -> p c h", p=128).bitcast(F32R))
        KBLOB = 2 * 128 * TC
        VBLOB = 5 * 128 * H
        for hh in range(2):
            kblob = bnc_out[hh, 0:KBLOB].rearrange(
                "(hc p t) -> p hc t", hc=2, p=128).bitcast(F32R)
            for hc in range(2):
                nc.sync.dma_start(
                    kT_sb[:, hc * S + SCC + hh * T0C:
                          hc * S + SCC + hh * T0C + T0C],
                    kblob[:, hc, 0:T0C])
                nc.sync.dma_start(
                    kT_sb[:, hc * S + SCC + T0 + hh * T1C:
                          hc * S + SCC + T0 + hh * T1C + T1C],
                    kblob[:, hc, T0C:TC])
            vblob = bnc_out[hh, KBLOB:KBLOB + VBLOB].rearrange(
                "(c p h) -> p c h", c=5, p=128).bitcast(F32R)
            nc.sync.dma_start(
                v_sb[:, (8 + hh * 4) * H: (8 + hh * 4) * H + 4 * H]
                .rearrange("p (c h) -> p c h", c=4),
                vblob[:, 0:4, :])
            nc.sync.dma_start(
                v_sb[:, (16 + hh) * H: (16 + hh) * H + H], vblob[:, 4, :])

        encs = [encT_pool.tile([128, 2 * TC], F32R, tag="encT",
                               name=f"encT_{n}") for n in range(NH)]

        # ---- attention (logitsT layout, no max-subtraction) ----
        with tc.tile_pool(name="maskp", bufs=12) as maskp, \
             tc.tile_pool(name="ep", bufs=6) as ep, \
             tc.tile_pool(name="recipp", bufs=2) as recipp, \
             tc.tile_pool(name="ps_lg", bufs=2, space="PSUM") as ps_lg, \
             tc.tile_pool(name="ps_enc", bufs=4, space="PSUM") as ps_enc, \
             tc.tile_pool(name="ps_den", bufs=2, space="PSUM") as ps_den:
            for tcn in range(N_TCH):
                tsl = slice(tcn * TCH, (tcn + 1) * TCH)
                active = [sc for sc in range(N_SCH) if cls[tcn][sc] != ZERO]
                mtiles = {}
                for sc in active:
                    if cls[tcn][sc] == MIXED:
                        mk = maskp.tile([128, TCH], F32, tag="mask",
                                        name=f"mk_{tcn}_{sc}")
                        nc.sync.dma_start(
                            mk[:], maskT_d[sc * 128:(sc + 1) * 128, tsl])
                        mtiles[sc] = mk
                for n in range(NH):
                    qT = qts[n]
                    enc0 = ps_enc.tile([128, TCH], F32, tag="enc",
                                       name=f"enc0_{tcn}_{n}")
                    enc1 = ps_enc.tile([128, TCH], F32, tag="enc",
                                       name=f"enc1_{tcn}_{n}")
                    den = ps_den.tile([128, TCH], F32, tag="den",
                                      name=f"den_{tcn}_{n}")
                    for i, sc in enumerate(active):
                        lg = ps_lg.tile([128, TCH], F32, tag="lg",
                                        name=f"lg_{tcn}_{n}_{sc}")
                        for hc in range(2):
                            nc.tensor.matmul(
                                lg[:],
                                kT_sb[:, hc * S + sc * 128:
                                      hc * S + sc * 128 + 128],
                                qT[:, hc * TC + tcn * TCH:
                                   hc * TC + tcn * TCH + TCH],
                                start=(hc == 0), stop=(hc == 1))
                        if sc in mtiles:
                            nc.vector.tensor_tensor(lg[:], lg[:],
                                                    mtiles[sc][:],
                                                    AluOpType.add)
                        eT = ep.tile([128, TCH], F32R, tag="eT",
                                     name=f"eT_{tcn}_{n}_{sc}")
                        nc.scalar.activation(eT[:], lg[:], AF.Exp)
                        first, last = (i == 0), (i == len(active) - 1)
                        for hc, enc in ((0, enc0), (1, enc1)):
                            nc.tensor.matmul(
                                enc[:],
                                v_sb[:, sc * H + hc * 128:
                                     sc * H + hc * 128 + 128],
                                eT[:], start=first, stop=last)
                        nc.tensor.matmul(den[:], ones_r[:], eT[:],
                                         start=first, stop=last)
                    recip = recipp.tile([128, TCH], F32, tag="recip",
                                        name=f"rc_{tcn}_{n}")
                    nc.vector.reciprocal(recip[:], den[:])
                    for hc, enc in ((0, enc0), (1, enc1)):
                        nc.vector.tensor_tensor(
                            encs[n][:, hc * TC + tcn * TCH:
                                    hc * TC + tcn * TCH + TCH],
                            enc[:], recip[:], AluOpType.mult)

        # ---- output projections ----
        with tc.tile_pool(name="wo", bufs=3) as wop, \
             tc.tile_pool(name="osb", bufs=4) as osb, \
             tc.tile_pool(name="ps_out", bufs=6, space="PSUM") as ps_out:
            for d in range(4):
                outp = [ps_out.tile([128, 512], F32, tag="po",
                                    name=f"po0_{d}_{t4}") for t4 in range(4)]
                for kc in range(16):
                    n, hc = kc // 2, kc % 2
                    rhs = wop.tile([128, 512], F32R, tag="wo",
                                   name=f"wo0_{d}_{kc}")
                    nc.sync.dma_start(
                        rhs[:],
                        wo0_d[kc * 128:(kc + 1) * 128,
                              d * 512:(d + 1) * 512].bitcast(F32R))
                    for t4 in range(4):
                        nc.tensor.matmul(
                            outp[t4][:],
                            encs[n][:, hc * TC + t4 * 128:
                                    hc * TC + t4 * 128 + 128],
                            rhs[:], start=(kc == 0), stop=(kc == 15))
                for t4 in range(4):
                    ot = osb.tile([128, 512], F32, tag="ot", name=f"ot0_{d}_{t4}")
                    nc.vector.tensor_copy(ot[:], outp[t4][:])
                    nc.sync.dma_start(
                        out0_d[t4 * 128:(t4 + 1) * 128, d * 512:(d + 1) * 512],
                        ot[:])
            for d in range(2):
                op1 = ps_out.tile([128, 512], F32, tag="po", name=f"po1_{d}")
                for kc in range(16):
                    n, hc = kc // 2, kc % 2
                    rhs = wop.tile([128, 512], F32R, tag="wo",
                                   name=f"wo1_{d}_{kc}")
                    nc.sync.dma_start(
                        rhs[:],
                        wo1_d[kc * 128:(kc + 1) * 128,
                              d * 512:(d + 1) * 512].bitcast(F32R))
                    nc.tensor.matmul(
                        op1[:],
                        encs[n][:, hc * TC + T0C: hc * TC + TC],
                        rhs[:], start=(kc == 0), stop=(kc == 15))
                ot = osb.tile([128, 512], F32, tag="ot", name=f"ot1_{d}")
                nc.vector.tensor_copy(ot[:], op1[:])
                nc.sync.dma_start(
                    out1_d[:, d * 512:(d + 1) * 512], ot[:])

    nc.compile()
    return nc


def _prep_inputs(inputs):
    """Host-side staging: slice/transpose per core; classify mask blocks."""
    x0 = np.ascontiguousarray(inputs["x0"], dtype=np.float32)
    x1 = np.ascontiguousarray(inputs["x1"], dtype=np.float32)
    wq0 = np.ascontiguousarray(inputs["wq0"], dtype=np.float32)
    wkv0 = np.ascontiguousarray(np.asarray(inputs["wkv0"], dtype=np.float32)[:, 0])
    wo0 = np.ascontiguousarray(inputs["wo0"], dtype=np.float32)
    wq1 = np.ascontiguousarray(inputs["wq1"], dtype=np.float32)
    wkv1 = np.ascontiguousarray(np.asarray(inputs["wkv1"], dtype=np.float32)[:, 0])
    wo1 = np.ascontiguousarray(inputs["wo1"], dtype=np.float32)
    cache_k = np.asarray(inputs["cache_k"], dtype=np.float32)[:, :, 0]
    cache_v = np.asarray(inputs["cache_v"], dtype=np.float32)[:, :, 0]
    positions = np.asarray(inputs["positions"], dtype=np.float32)
    mask = np.asarray(inputs["attn_mask"])[:, 0]          # [B, T, S] bool

    wo0f = np.ascontiguousarray(wo0.reshape(NH * H, W0))
    wo1f = np.ascontiguousarray(wo1.reshape(NH * H, W1))
    half = H // 2
    invts = (10000.0 ** (-(2.0 / H) * np.arange(half, dtype=np.float32))
             ).astype(np.float32).reshape(1, half)
    ident = np.eye(128, dtype=np.float32)

    in_maps = []
    maskTs = []
    for c in range(N_CORES):
        b, h = divmod(c, 2)
        sl0 = slice(h * T0C, (h + 1) * T0C)
        sl1 = slice(T0 + h * T1C, T0 + (h + 1) * T1C)
        x0T = np.ascontiguousarray(x0[b, sl0].T)          # [W0, 512]
        x1T = np.ascontiguousarray(x1[b, h * T1C:(h + 1) * T1C].T)
        pos = np.concatenate([positions[b, sl0], positions[b, sl1]]
                             ).reshape(1, TC).astype(np.float32)
        m_rows = np.concatenate([mask[b, sl0], mask[b, sl1]], axis=0)
        maskTs.append(m_rows.T)                           # [S, 640] bool
        maskT = np.where(m_rows.T, np.float32(0.0), np.float32(BIG_NEG))
        ckT = np.ascontiguousarray(cache_k[b].T)          # [H, 1024]
        in_maps.append({
            "x0T": x0T, "x1T": x1T,
            "wq0": wq0, "wkv0": wkv0, "wq1": wq1, "wkv1": wkv1,
            "wo0f": wo0f, "wo1f": wo1f,
            "ckT": ckT, "cv": np.ascontiguousarray(cache_v[b]),
            "pos": pos, "invts": invts, "ident": ident,
            "maskT": np.ascontiguousarray(maskT),
        })

    allm = np.stack(maskTs)                               # [8, S, 640] bool
    cls = []
    for tcn in range(N_TCH):
        row = []
        for sc in range(N_SCH):
            blk = allm[:, sc * 128:(sc + 1) * 128,
                       tcn * TCH:(tcn + 1) * TCH]
            if blk.all():
                row.append(ONES)
            elif not blk.any():
                row.append(ZERO)
            else:
                row.append(MIXED)
        cls.append(tuple(row))
    return in_maps, tuple(cls)


def kernel(**inputs):
    in_maps, cls = _prep_inputs(inputs)
    if cls not in _CACHE:
        _CACHE[cls] = _build(cls)
    nc = _CACHE[cls]
    res = run_bass_kernel_spmd(nc, in_maps, core_ids=list(range(N_CORES)))

    out0 = np.empty((B, T0, W0), dtype=np.float32)
    out1 = np.empty((B, T1, W1), dtype=np.float32)
    k = np.empty((B, S, 1, H), dtype=np.float32)
    v = np.empty((B, S, 1, H), dtype=np.float32)
    k[:, :SCC] = np.asarray(inputs["cache_k"], dtype=np.float32)
    v[:, :SCC] = np.asarray(inputs["cache_v"], dtype=np.float32)
    for c in range(N_CORES):
        b, h = divmod(c, 2)
        r = res.results[c]
        out0[b, h * T0C:(h + 1) * T0C] = r["out0"]
        out1[b, h * T1C:(h + 1) * T1C] = r["out1"]
        kTn = r["kTn"]                                    # [H, 640]
        vn = r["vn"]                                      # [640, H]
        k[b, SCC + h * T0C: SCC + (h + 1) * T0C, 0] = kTn[:, :T0C].T
        k[b, SCC + T0 + h * T1C: SCC + T0 + (h + 1) * T1C, 0] = kTn[:, T0C:].T
        v[b, SCC + h * T0C: SCC + (h + 1) * T0C, 0] = vn[:T0C]
        v[b, SCC + T0 + h * T1C: SCC + T0 + (h + 1) * T1C, 0] = vn[T0C:]
    return out0, out1, k, v


# revision 19
# speedup vs baseline: 1.0628x; 1.0628x over previous
"""Trainium2 Bass kernel for MQA attention with per-expert QKV/out projections.

Problem (hardcoded shapes):
  B=4, T0=1024 (W0=2048 expert), T1=256 (W1=1024 expert), cache 1024, S=2304,
  N=8 query heads, KV=1 (MQA), H=256.  Returns (out0, out1, k, v).

Sharding: 8 cores = (batch b, token-half h): core c -> b=c//2, h=c%2.
Each core projects q/k/v for its 640 tokens (512 from x0, 128 from x1),
all-gathers k/v with its batch peer, runs attention for all 8 heads over
the full 2304-key axis in transposed (logitsT = [S, T]) layout, and applies
the output projections for its tokens.

Matmul operands are cast (on device) to fp16: fp32/fp32r moving operands
stream at ~2 cycles/row on the PE and their 4-byte weight loads (~200 ns,
no fast-weight-load) stall the array; fp16 runs at 1 cycle/row with FWL.
PSUM accumulation is fp32 throughout; k/v outputs are computed in fp32.

The attention mask is handled per 128(s) x 320(t) block, classified from the
actual input (union over cores): fully-masked blocks are skipped, fully-open
blocks need no mask op, mixed blocks get an additive mask loaded from DRAM.
"""
import numpy as np
from contextlib import ExitStack

import concourse.bacc as bacc
import concourse.tile as tile
import concourse.mybir as mybir
from concourse.alu_op_type import AluOpType
from concourse.bass_utils import run_bass_kernel_spmd

F32 = mybir.dt.float32
F16 = mybir.dt.float16
AF = mybir.ActivationFunctionType
PI = float(np.pi)
MAGIC = float(2**23)

B, T0, T1, SCC = 4, 1024, 256, 1024
T = T0 + T1                 # 1280
S = SCC + T                 # 2304
W0, W1 = 2048, 1024
NH, H = 8, 256
SCALE = float(H**-0.5)      # 1/16
EXP_SHIFT = -4.0            # softmax-invariant shift keeping exp in fp16 range

N_CORES = 8
T0C, T1C = T0 // 2, T1 // 2  # 512, 128 per core
TC = T0C + T1C               # 640
TCH = 320                    # attention t-chunk
N_TCH = TC // TCH            # 2
N_SCH = S // 128             # 18 s-chunks
BIG_NEG = -1.0e30
ZERO, ONES, MIXED = 0, 1, 2

_CACHE = {}


def _range_reduce_sin_cos(nc, pool, rad_ps, sin_dst, cos_dst, width):
    """From radians psum tile [128, width], write sin/cos into dst APs.

    sin arg reduced to [-pi, pi] via r - 2pi*round(r/2pi) (fp32 magic-round);
    cos arg reduced via r + pi/2 - 2pi*round(r/2pi + 0.25).
    """
    a = pool.tile([128, width], F32, tag="rr_a", name="rr_a")
    nc.vector.tensor_scalar(a[:], rad_ps[:], 1.0 / (2 * PI), MAGIC,
                            AluOpType.mult, AluOpType.add)
    b = pool.tile([128, width], F32, tag="rr_b", name="rr_b")
    nc.vector.tensor_scalar(b[:], a[:], MAGIC, -2 * PI,
                            AluOpType.subtract, AluOpType.mult)
    m = pool.tile([128, width], F32, tag="rr_c", name="rr_c")
    nc.vector.tensor_tensor(m[:], rad_ps[:], b[:], AluOpType.add)
    nc.scalar.activation(sin_dst, m[:], AF.Sin)

    a2 = pool.tile([128, width], F32, tag="rr_a", name="rr_a2")
    nc.vector.tensor_scalar(a2[:], rad_ps[:], 1.0 / (2 * PI), 0.25,
                            AluOpType.mult, AluOpType.add)
    b2 = pool.tile([128, width], F32, tag="rr_b", name="rr_b2")
    nc.vector.tensor_scalar(b2[:], a2[:], MAGIC, MAGIC,
                            AluOpType.add, AluOpType.subtract)
    c2 = pool.tile([128, width], F32, tag="rr_c", name="rr_c2")
    nc.vector.tensor_scalar(c2[:], b2[:], -2 * PI, PI / 2,
                            AluOpType.mult, AluOpType.add)
    m2 = pool.tile([128, width], F32, tag="rr_d", name="rr_d2")
    nc.vector.tensor_tensor(m2[:], rad_ps[:], c2[:], AluOpType.add)
    nc.scalar.activation(cos_dst, m2[:], AF.Sin)


def _rope_pair(nc, pool, out0_ap, out1_ap, p0, p1, ss, cc, width, tag):
    """out0 = p0*cc - p1*ss ; out1 = p1*cc + p0*ss (elementwise [128, width])."""
    t1 = pool.tile([128, width], F32, tag="rope_t1", name=f"rt1_{tag}")
    nc.vector.tensor_tensor(t1[:], p0, cc, AluOpType.mult)
    t2 = pool.tile([128, width], F32, tag="rope_t2", name=f"rt2_{tag}")
    nc.vector.tensor_tensor(t2[:], p1, ss, AluOpType.mult)
    nc.vector.tensor_tensor(out0_ap, t1[:], t2[:], AluOpType.subtract)
    t3 = pool.tile([128, width], F32, tag="rope_t1", name=f"rt3_{tag}")
    nc.vector.tensor_tensor(t3[:], p1, cc, AluOpType.mult)
    t4 = pool.tile([128, width], F32, tag="rope_t2", name=f"rt4_{tag}")
    nc.vector.tensor_tensor(t4[:], p0, ss, AluOpType.mult)
    nc.vector.tensor_tensor(out1_ap, t3[:], t4[:], AluOpType.add)


def _build(cls):
    """cls[tcn][sc] in {ZERO, ONES, MIXED} — mask block classification."""
    nc = bacc.Bacc(trn_type="TRN2", debug=False, num_devices=N_CORES)

    def din(name, shape):
        return nc.dram_tensor(name, shape, F32, kind="ExternalInput").ap()

    def dout(name, shape):
        return nc.dram_tensor(name, shape, F32, kind="ExternalOutput").ap()

    x0T_d = din("x0T", [W0, T0C])          # this core's x0 tokens, transposed
    x1T_d = din("x1T", [W1, T1C])
    wq0_d = din("wq0", [NH, W0, H])
    wkv0_d = din("wkv0", [2, W0, H])
    wq1_d = din("wq1", [NH, W1, H])
    wkv1_d = din("wkv1", [2, W1, H])
    wo0_d = din("wo0f", [NH * H, W0])      # flattened (n,h)-major
    wo1_d = din("wo1f", [NH * H, W1])
    ckT_d = din("ckT", [H, SCC])           # cache_k[b] transposed
    cv_d = din("cv", [SCC, H])             # cache_v[b]
    pos_d = din("pos", [1, TC])            # positions of this core's tokens
    invts_d = din("invts", [1, 128])       # 1/timescale
    ident_d = din("ident", [128, 128])     # identity for PE transpose
    maskT_d = din("maskT", [S, TC])        # additive mask, [s, t] layout

    out0_d = dout("out0", [T0C, W0])
    out1_d = dout("out1", [T1C, W1])
    kTn_d = dout("kTn", [H, TC])           # roped new k, [H, t]
    vn_d = dout("vn", [TC, H])             # new v, [t, H]

    with tile.TileContext(nc, num_cores=N_CORES) as tc, ExitStack() as ctx:
        # ---- whole-kernel pools ----
        pers = ctx.enter_context(tc.tile_pool(name="pers", bufs=1))
        qT_pool = ctx.enter_context(tc.tile_pool(name="qT", bufs=NH))
        encT_pool = ctx.enter_context(tc.tile_pool(name="encT", bufs=NH))
        dram = ctx.enter_context(tc.tile_pool(name="dram", bufs=1, space="DRAM"))

        ident = pers.tile([128, 128], F32)
        nc.sync.dma_start(ident[:], ident_d[:])
        ones_f = pers.tile([128, 128], F32)
        nc.vector.memset(ones_f[:], 1.0)
        ones_16 = pers.tile([128, 128], F16)
        nc.vector.tensor_copy(ones_16[:], ones_f[:])
        neg4 = pers.tile([128, 1], F32)
        nc.vector.memset(neg4[:], EXP_SHIFT)
        pos_sb = pers.tile([1, TC], F32)
        nc.sync.dma_start(pos_sb[:], pos_d[:])
        invts_sb = pers.tile([1, 128], F32)
        nc.sync.dma_start(invts_sb[:], invts_d[:])

        # rope tables ([i, t] layout for H-major rope; [t, i] for x1 path)
        sinT = pers.tile([128, TC], F32)
        cosT = pers.tile([128, TC], F32)
        sinTq = pers.tile([128, TC], F32)
        cosTq = pers.tile([128, TC], F32)
        sinF = pers.tile([128, 128], F32)
        cosF = pers.tile([128, 128], F32)
        sinFq = pers.tile([128, 128], F32)
        cosFq = pers.tile([128, 128], F32)
        with tc.tile_pool(name="ps_rad", bufs=2, space="PSUM") as ps_rad, \
             tc.tile_pool(name="rrtmp", bufs=2) as rrtmp:
            for half in range(2):
                sl = slice(half * TCH, (half + 1) * TCH)
                rad = ps_rad.tile([128, TCH], F32, tag="rad", name=f"radT{half}")
                nc.tensor.matmul(rad[:], invts_sb[:], pos_sb[:, sl],
                                 start=True, stop=True)
                _range_reduce_sin_cos(nc, rrtmp, rad, sinT[:, sl],
                                      cosT[:, sl], TCH)
            radF = ps_rad.tile([128, 128], F32, tag="rad", name="radF")
            nc.tensor.matmul(radF[:], pos_sb[:, T0C:TC], invts_sb[:],
                             start=True, stop=True)
            _range_reduce_sin_cos(nc, rrtmp, radF, sinF[:], cosF[:], 128)
        for src, dst in ((sinT, sinTq), (cosT, cosTq), (sinF, sinFq),
                         (cosF, cosFq)):
            nc.vector.tensor_scalar(dst[:], src[:], SCALE, None, AluOpType.mult)

        qts = [qT_pool.tile([128, 2 * TC], F16, tag="qT", name=f"qT_{n}")
               for n in range(NH)]

        # ---- projections ----
        with tc.tile_pool(name="xstage", bufs=1) as xstage, \
             tc.tile_pool(name="kvtmp", bufs=1) as kvtmp, \
             tc.tile_pool(name="tmp", bufs=2) as tmp_pool, \
             tc.tile_pool(name="wstage", bufs=3) as wstage, \
             tc.tile_pool(name="wbig", bufs=3) as wbig, \
             tc.tile_pool(name="wsmall", bufs=2) as wsmall, \
             tc.tile_pool(name="ps_qp", bufs=3, space="PSUM") as ps_qp, \
             tc.tile_pool(name="ps_vp", bufs=2, space="PSUM") as ps_vp, \
             tc.tile_pool(name="ps_tr", bufs=2, space="PSUM") as ps_tr:

            # x activations: DMA fp32 chunks, cast to one resident fp16 tile
            x0T_sb = xstage.tile([128, 16 * T0C], F16)   # [128, (dc t)]
            x0_src = x0T_d.rearrange("(c p) t -> p c t", p=128)
            for dc in range(16):
                stg = wstage.tile([128, T0C], F32, tag="wstage",
                                  name=f"x0stg_{dc}")
                nc.sync.dma_start(stg[:], x0_src[:, dc, :])
                nc.vector.tensor_copy(
                    x0T_sb[:, dc * T0C:(dc + 1) * T0C], stg[:])
            x1T_sb = xstage.tile([128, 8 * T1C], F16)
            x1_src = x1T_d.rearrange("(c p) t -> p c t", p=128)
            for dc2 in range(4):   # two 128-d chunks per stage tile
                stg = wstage.tile([128, 2 * T1C], F32, tag="x1stage",
                                  name=f"x1stg_{dc2}")
                nc.sync.dma_start(
                    stg[:].rearrange("p (c t) -> p c t", c=2),
                    x1_src[:, 2 * dc2: 2 * dc2 + 2, :])
                nc.vector.tensor_copy(
                    x1T_sb[:, 2 * dc2 * T1C:(2 * dc2 + 2) * T1C], stg[:])

            ktmp = kvtmp.tile([128, 2 * TC], F32)    # new k, [hc, t] free-major
            vtmp = kvtmp.tile([128, 5 * 256], F32)   # new v, 5 token-chunks

            def load_w16(w_ap, name, cast_engine="vector"):
                """[2048, 256] fp32 weight -> two [128, 8*256] fp16 tiles."""
                src = w_ap.rearrange("(c p) h -> p c h", p=128)
                tiles = []
                for hf in range(2):
                    stg = wstage.tile([128, 8 * H], F32, tag="wstage",
                                      name=f"{name}_s{hf}")
                    nc.sync.dma_start(
                        stg[:].rearrange("p (c h) -> p c h", c=8),
                        src[:, hf * 8:(hf + 1) * 8, :])
                    wt = wbig.tile([128, 8 * H], F16, tag="wbig",
                                   name=f"{name}_{hf}")
                    if cast_engine == "vector":
                        nc.vector.tensor_copy(wt[:], stg[:])
                    else:
                        nc.scalar.activation(wt[:], stg[:], AF.Copy)
                    tiles.append(wt)

                def sl(dc, off=0, width=H):
                    base = (dc % 8) * H + off
                    return tiles[dc // 8][:, base: base + width]
                return sl

            # x0 -> q (per head, [H,t] layout), via lhsT=wq0 chunks
            for n in range(NH):
                wq = load_w16(wq0_d[n], f"wq0_{n}", "scalar")
                qps = []
                for hc in range(2):
                    qp = ps_qp.tile([128, T0C], F32, tag="qp",
                                    name=f"qp_{n}_{hc}")
                    for dc in range(16):
                        nc.tensor.matmul(
                            qp[:],
                            wq(dc, hc * 128, 128),
                            x0T_sb[:, dc * T0C: (dc + 1) * T0C],
                            start=(dc == 0), stop=(dc == 15))
                    qps.append(qp)
                _rope_pair(nc, tmp_pool, qts[n][:, 0:T0C],
                           qts[n][:, TC:TC + T0C], qps[0][:], qps[1][:],
                           sinTq[:, 0:T0C], cosTq[:, 0:T0C], T0C, f"q{n}")

            # x0 -> k ([H,t] layout)
            wk = load_w16(wkv0_d[0], "wk0", "scalar")
            kps = []
            for hc in range(2):
                kp = ps_qp.tile([128, T0C], F32, tag="qp", name=f"kp_{hc}")
                for dc in range(16):
                    nc.tensor.matmul(
                        kp[:],
                        wk(dc, hc * 128, 128),
                        x0T_sb[:, dc * T0C: (dc + 1) * T0C],
                        start=(dc == 0), stop=(dc == 15))
                kps.append(kp)
            _rope_pair(nc, tmp_pool, ktmp[:, 0:T0C], ktmp[:, TC:TC + T0C],
                       kps[0][:], kps[1][:], sinT[:, 0:T0C], cosT[:, 0:T0C],
                       T0C, "k")

            # x0 -> v ([t, H] layout)
            wv = load_w16(wkv0_d[1], "wv0", "scalar")
            for tcn in range(4):
                vp = ps_vp.tile([128, H], F32, tag="vp", name=f"vp_{tcn}")
                for dc in range(16):
                    nc.tensor.matmul(
                        vp[:],
                        x0T_sb[:, dc * T0C + tcn * 128: dc * T0C + tcn * 128 + 128],
                        wv(dc),
                        start=(dc == 0), stop=(dc == 15))
                nc.vector.tensor_copy(vtmp[:, tcn * H: (tcn + 1) * H], vp[:])

            # x1 -> q/k in [t, H] then PE-transpose into [H, t]
            def x1_proj(w_d, name):
                src = w_d.rearrange("(c p) h -> p c h", p=128)
                stg = wstage.tile([128, 8 * H], F32, tag="wstage",
                                  name=f"ws_{name}")
                nc.sync.dma_start(
                    stg[:].rearrange("p (c h) -> p c h", c=8), src)
                ww = wsmall.tile([128, 8 * H], F16, tag="wsmall",
                                 name=f"w_{name}")
                nc.scalar.activation(ww[:], stg[:], AF.Copy)
                pp = ps_vp.tile([128, H], F32, tag="vp", name=f"pp_{name}")
                for dc in range(8):
                    nc.tensor.matmul(
                        pp[:],
                        x1T_sb[:, dc * T1C: (dc + 1) * T1C],
                        ww[:, dc * H: (dc + 1) * H],
                        start=(dc == 0), stop=(dc == 7))
                return pp

            def rope_F(pp, scaled, name):
                ss, cc = (sinFq, cosFq) if scaled else (sinF, cosF)
                ro = tmp_pool.tile([128, H], F32, tag="ropeF", name=f"ro_{name}")
                _rope_pair(nc, tmp_pool, ro[:, 0:128], ro[:, 128:256],
                           pp[:, 0:128], pp[:, 128:256], ss[:], cc[:], 128,
                           name)
                return ro

            def transpose_to(ro, out_tile, base_off, name):
                for hc in range(2):
                    tp = ps_tr.tile([128, 128], F32, tag="tr",
                                    name=f"tp_{name}_{hc}")
                    nc.tensor.transpose(tp[:], ro[:, hc * 128:(hc + 1) * 128],
                                        ident[:])
                    nc.vector.tensor_copy(
                        out_tile[:, base_off + hc * TC:
                                 base_off + hc * TC + T1C], tp[:])

            for n in range(NH):
                pp = x1_proj(wq1_d[n], f"q1_{n}")
                ro = rope_F(pp, True, f"q1_{n}")
                transpose_to(ro, qts[n], T0C, f"q1_{n}")

            pp = x1_proj(wkv1_d[0], "k1")
            ro = rope_F(pp, False, "k1")
            # k1 transpose goes into fp32 ktmp
            for hc in range(2):
                tp = ps_tr.tile([128, 128], F32, tag="tr", name=f"tp_k1_{hc}")
                nc.tensor.transpose(tp[:], ro[:, hc * 128:(hc + 1) * 128],
                                    ident[:])
                nc.vector.tensor_copy(
                    ktmp[:, T0C + hc * TC: T0C + hc * TC + T1C], tp[:])

            pp = x1_proj(wkv1_d[1], "v1")
            nc.vector.tensor_copy(vtmp[:, 4 * H: 5 * H], pp[:])

            # ---- fp32 outputs for new k/v ----
            for hc in range(2):
                nc.sync.dma_start(kTn_d[hc * 128:(hc + 1) * 128, :],
                                  ktmp[:, hc * TC:(hc + 1) * TC])
            nc.sync.dma_start(vn_d.rearrange("(c p) h -> p c h", p=128),
                              vtmp[:].rearrange("p (c h) -> p c h", c=5))

            # ---- fp16 copies + allgather ----
            kt16 = kvtmp.tile([128, 2 * TC], F16)
            nc.vector.tensor_copy(kt16[:], ktmp[:])
            vt16 = kvtmp.tile([128, 5 * 256], F16)
            nc.vector.tensor_copy(vt16[:], vtmp[:])

            KBLOB = 2 * 128 * TC        # fp16 elems
            VBLOB = 5 * 128 * H
            bnc_in = dram.tile([KBLOB + VBLOB], F16)
            bnc_out = dram.tile([2, KBLOB + VBLOB], F16)
            nc.sync.dma_start(
                bnc_in[0:KBLOB].rearrange("(hc p t) -> p hc t", hc=2, p=128),
                kt16[:].rearrange("p (hc t) -> p hc t", hc=2))
            nc.sync.dma_start(
                bnc_in[KBLOB:KBLOB + VBLOB].rearrange("(c p h) -> p c h",
                                                      c=5, p=128),
                vt16[:].rearrange("p (c h) -> p c h", c=5))
            nc.gpsimd.collective_compute(
                "AllGather", mybir.AluOpType.bypass,
                replica_groups=[[0, 1], [2, 3], [4, 5], [6, 7]],
                ins=[bnc_in[:]], outs=[bnc_out[:]])

        # ---- assemble kT [128, (hc s)] and v [128, (sc h)] in fp16 ----
        kv_pool = ctx.enter_context(tc.tile_pool(name="kv", bufs=1))
        kT_sb = kv_pool.tile([128, 2 * S], F16)
        v_sb = kv_pool.tile([128, N_SCH * H], F16)
        with tc.tile_pool(name="cstage", bufs=2) as cstage:
            for hc in range(2):
                stg = cstage.tile([128, SCC], F32, tag="cstage",
                                  name=f"ck_{hc}")
                nc.sync.dma_start(stg[:], ckT_d[hc * 128:(hc + 1) * 128, :])
                nc.vector.tensor_copy(kT_sb[:, hc * S: hc * S + SCC], stg[:])
            for cvh in range(2):
                stg = cstage.tile([128, SCC], F32, tag="cstage",
                                  name=f"cv_{cvh}")
                nc.sync.dma_start(
                    stg[:].rearrange("p (c h) -> p c h", c=4),
                    cv_d.rearrange("(c p) h -> p c h", p=128)[:, cvh * 4:
                                                             cvh * 4 + 4, :])
                nc.vector.tensor_copy(
                    v_sb[:, cvh * 4 * H: (cvh * 4 + 4) * H], stg[:])
        KBLOB = 2 * 128 * TC
        VBLOB = 5 * 128 * H
        for hh in range(2):
            kblob = bnc_out[hh, 0:KBLOB].rearrange(
                "(hc p t) -> p hc t", hc=2, p=128)
            for hc in range(2):
                nc.sync.dma_start(
                    kT_sb[:, hc * S + SCC + hh * T0C:
                          hc * S + SCC + hh * T0C + T0C],
                    kblob[:, hc, 0:T0C])
                nc.sync.dma_start(
                    kT_sb[:, hc * S + SCC + T0 + hh * T1C:
                          hc * S + SCC + T0 + hh * T1C + T1C],
                    kblob[:, hc, T0C:TC])
            vblob = bnc_out[hh, KBLOB:KBLOB + VBLOB].rearrange(
                "(c p h) -> p c h", c=5, p=128)
            nc.sync.dma_start(
                v_sb[:, (8 + hh * 4) * H: (8 + hh * 4) * H + 4 * H]
                .rearrange("p (c h) -> p c h", c=4),
                vblob[:, 0:4, :])
            nc.sync.dma_start(
                v_sb[:, (16 + hh) * H: (16 + hh) * H + H], vblob[:, 4, :])

        encs = [encT_pool.tile([128, 2 * TC], F16, tag="encT",
                               name=f"encT_{n}") for n in range(NH)]

        # ---- attention (logitsT layout, shifted exp, no max-subtraction) ----
        with tc.tile_pool(name="maskp", bufs=12) as maskp, \
             tc.tile_pool(name="ep", bufs=6) as ep, \
             tc.tile_pool(name="recipp", bufs=2) as recipp, \
             tc.tile_pool(name="ps_lg", bufs=2, space="PSUM") as ps_lg, \
             tc.tile_pool(name="ps_enc", bufs=4, space="PSUM") as ps_enc, \
             tc.tile_pool(name="ps_den", bufs=2, space="PSUM") as ps_den:
            for tcn in range(N_TCH):
                tsl = slice(tcn * TCH, (tcn + 1) * TCH)
                active = [sc for sc in range(N_SCH) if cls[tcn][sc] != ZERO]
                mtiles = {}
                for sc in active:
                    if cls[tcn][sc] == MIXED:
                        mk = maskp.tile([128, TCH], F32, tag="mask",
                                        name=f"mk_{tcn}_{sc}")
                        nc.sync.dma_start(
                            mk[:], maskT_d[sc * 128:(sc + 1) * 128, tsl])
                        mtiles[sc] = mk
                for n in range(NH):
                    qT = qts[n]
                    enc0 = ps_enc.tile([128, TCH], F32, tag="enc",
                                       name=f"enc0_{tcn}_{n}")
                    enc1 = ps_enc.tile([128, TCH], F32, tag="enc",
                                       name=f"enc1_{tcn}_{n}")
                    den = ps_den.tile([128, TCH], F32, tag="den",
                                      name=f"den_{tcn}_{n}")
                    for i, sc in enumerate(active):
                        lg = ps_lg.tile([128, TCH], F32, tag="lg",
                                        name=f"lg_{tcn}_{n}_{sc}")
                        for hc in range(2):
                            nc.tensor.matmul(
                                lg[:],
                                kT_sb[:, hc * S + sc * 128:
                                      hc * S + sc * 128 + 128],
                                qT[:, hc * TC + tcn * TCH:
                                   hc * TC + tcn * TCH + TCH],
                                start=(hc == 0), stop=(hc == 1))
                        if sc in mtiles:
                            nc.vector.tensor_tensor(lg[:], lg[:],
                                                    mtiles[sc][:],
                                                    AluOpType.add)
                        eT = ep.tile([128, TCH], F16, tag="eT",
                                     name=f"eT_{tcn}_{n}_{sc}")
                        nc.scalar.activation(eT[:], lg[:], AF.Exp,
                                             bias=neg4[:])
                        first, last = (i == 0), (i == len(active) - 1)
                        for hc, enc in ((0, enc0), (1, enc1)):
                            nc.tensor.matmul(
                                enc[:],
                                v_sb[:, sc * H + hc * 128:
                                     sc * H + hc * 128 + 128],
                                eT[:], start=first, stop=last)
                        nc.tensor.matmul(den[:], ones_16[:], eT[:],
                                         start=first, stop=last)
                    recip = recipp.tile([128, TCH], F32, tag="recip",
                                        name=f"rc_{tcn}_{n}")
                    nc.vector.reciprocal(recip[:], den[:])
                    for hc, enc in ((0, enc0), (1, enc1)):
                        nc.vector.tensor_tensor(
                            encs[n][:, hc * TC + tcn * TCH:
                                    hc * TC + tcn * TCH + TCH],
                            enc[:], recip[:], AluOpType.mult)

        # ---- output projections ----
        with tc.tile_pool(name="wostage", bufs=3) as wostage, \
             tc.tile_pool(name="wo", bufs=3) as wop, \
             tc.tile_pool(name="osb", bufs=4) as osb, \
             tc.tile_pool(name="ps_out", bufs=6, space="PSUM") as ps_out:

            def load_wo(w_d, kc, d, name):
                stg = wostage.tile([128, 512], F32, tag="wos",
                                   name=f"s_{name}")
                nc.sync.dma_start(
                    stg[:], w_d[kc * 128:(kc + 1) * 128, d * 512:(d + 1) * 512])
                rhs = wop.tile([128, 512], F16, tag="wo", name=name)
                nc.scalar.activation(rhs[:], stg[:], AF.Copy)
                return rhs

            for d in range(4):
                outp = [ps_out.tile([128, 512], F32, tag="po",
                                    name=f"po0_{d}_{t4}") for t4 in range(4)]
                for kc in range(16):
                    n, hc = kc // 2, kc % 2
                    rhs = load_wo(wo0_d, kc, d, f"wo0_{d}_{kc}")
                    for t4 in range(4):
                        nc.tensor.matmul(
                            outp[t4][:],
                            encs[n][:, hc * TC + t4 * 128:
                                    hc * TC + t4 * 128 + 128],
                            rhs[:], start=(kc == 0), stop=(kc == 15))
                for t4 in range(4):
                    ot = osb.tile([128, 512], F32, tag="ot", name=f"ot0_{d}_{t4}")
                    nc.vector.tensor_copy(ot[:], outp[t4][:])
                    nc.sync.dma_start(
                        out0_d[t4 * 128:(t4 + 1) * 128, d * 512:(d + 1) * 512],
                        ot[:])
            for d in range(2):
                op1 = ps_out.tile([128, 512], F32, tag="po", name=f"po1_{d}")
                for kc in range(16):
                    n, hc = kc // 2, kc % 2
                    rhs = load_wo(wo1_d, kc, d, f"wo1_{d}_{kc}")
                    nc.tensor.matmul(
                        op1[:],
                        encs[n][:, hc * TC + T0C: hc * TC + TC],
                        rhs[:], start=(kc == 0), stop=(kc == 15))
                ot = osb.tile([128, 512], F32, tag="ot", name=f"ot1_{d}")
                nc.vector.tensor_copy(ot[:], op1[:])
                nc.sync.dma_start(
                    out1_d[:, d * 512:(d + 1) * 512], ot[:])

    nc.compile()
    return nc


def _prep_inputs(inputs):
    """Host-side staging: slice/transpose per core; classify mask blocks."""
    x0 = np.ascontiguousarray(inputs["x0"], dtype=np.float32)
    x1 = np.ascontiguousarray(inputs["x1"], dtype=np.float32)
    wq0 = np.ascontiguousarray(inputs["wq0"], dtype=np.float32)
    wkv0 = np.ascontiguousarray(np.asarray(inputs["wkv0"], dtype=np.float32)[:, 0])
    wo0 = np.ascontiguousarray(inputs["wo0"], dtype=np.float32)
    wq1 = np.ascontiguousarray(inputs["wq1"], dtype=np.float32)
    wkv1 = np.ascontiguousarray(np.asarray(inputs["wkv1"], dtype=np.float32)[:, 0])
    wo1 = np.ascontiguousarray(inputs["wo1"], dtype=np.float32)
    cache_k = np.asarray(inputs["cache_k"], dtype=np.float32)[:, :, 0]
    cache_v = np.asarray(inputs["cache_v"], dtype=np.float32)[:, :, 0]
    positions = np.asarray(inputs["positions"], dtype=np.float32)
    mask = np.asarray(inputs["attn_mask"])[:, 0]          # [B, T, S] bool

    wo0f = np.ascontiguousarray(wo0.reshape(NH * H, W0))
    wo1f = np.ascontiguousarray(wo1.reshape(NH * H, W1))
    half = H // 2
    invts = (10000.0 ** (-(2.0 / H) * np.arange(half, dtype=np.float32))
             ).astype(np.float32).reshape(1, half)
    ident = np.eye(128, dtype=np.float32)

    in_maps = []
    maskTs = []
    for c in range(N_CORES):
        b, h = divmod(c, 2)
        sl0 = slice(h * T0C, (h + 1) * T0C)
        sl1 = slice(T0 + h * T1C, T0 + (h + 1) * T1C)
        x0T = np.ascontiguousarray(x0[b, sl0].T)          # [W0, 512]
        x1T = np.ascontiguousarray(x1[b, h * T1C:(h + 1) * T1C].T)
        pos = np.concatenate([positions[b, sl0], positions[b, sl1]]
                             ).reshape(1, TC).astype(np.float32)
        m_rows = np.concatenate([mask[b, sl0], mask[b, sl1]], axis=0)
        maskTs.append(m_rows.T)                           # [S, 640] bool
        maskT = np.where(m_rows.T, np.float32(0.0), np.float32(BIG_NEG))
        ckT = np.ascontiguousarray(cache_k[b].T)          # [H, 1024]
        in_maps.append({
            "x0T": x0T, "x1T": x1T,
            "wq0": wq0, "wkv0": wkv0, "wq1": wq1, "wkv1": wkv1,
            "wo0f": wo0f, "wo1f": wo1f,
            "ckT": ckT, "cv": np.ascontiguousarray(cache_v[b]),
            "pos": pos, "invts": invts, "ident": ident,
            "maskT": np.ascontiguousarray(maskT),
        })

    allm = np.stack(maskTs)                               # [8, S, 640] bool
    cls = []
    for tcn in range(N_TCH):
        row = []
        for sc in range(N_SCH):
            blk = allm[:, sc * 128:(sc + 1) * 128,
                       tcn * TCH:(tcn + 1) * TCH]
            if blk.all():
                row.append(ONES)
            elif not blk.any():
                row.append(ZERO)
            else:
                row.append(MIXED)
        cls.append(tuple(row))
    return in_maps, tuple(cls)


def kernel(**inputs):
    in_maps, cls = _prep_inputs(inputs)
    if cls not in _CACHE:
        _CACHE[cls] = _build(cls)
    nc = _CACHE[cls]
    res = run_bass_kernel_spmd(nc, in_maps, core_ids=list(range(N_CORES)))

    out0 = np.empty((B, T0, W0), dtype=np.float32)
    out1 = np.empty((B, T1, W1), dtype=np.float32)
    k = np.empty((B, S, 1, H), dtype=np.float32)
    v = np.empty((B, S, 1, H), dtype=np.float32)
    k[:, :SCC] = np.asarray(inputs["cache_k"], dtype=np.float32)
    v[:, :SCC] = np.asarray(inputs["cache_v"], dtype=np.float32)
    for c in range(N_CORES):
        b, h = divmod(c, 2)
        r = res.results[c]
        out0[b, h * T0C:(h + 1) * T0C] = r["out0"]
        out1[b, h * T1C:(h + 1) * T1C] = r["out1"]
        kTn = r["kTn"]                                    # [H, 640]
        vn = r["vn"]                                      # [640, H]
        k[b, SCC + h * T0C: SCC + (h + 1) * T0C, 0] = kTn[:, :T0C].T
        k[b, SCC + T0 + h * T1C: SCC + T0 + (h + 1) * T1C, 0] = kTn[:, T0C:].T
        v[b, SCC + h * T0C: SCC + (h + 1) * T0C, 0] = vn[:T0C]
        v[b, SCC + T0 + h * T1C: SCC + T0 + (h + 1) * T1C, 0] = vn[T0C:]
    return out0, out1, k, v


# revision 20
# speedup vs baseline: 1.1428x; 1.0752x over previous
"""Trainium2 Bass kernel for MQA attention with per-expert QKV/out projections.

Problem (hardcoded shapes):
  B=4, T0=1024 (W0=2048 expert), T1=256 (W1=1024 expert), cache 1024, S=2304,
  N=8 query heads, KV=1 (MQA), H=256.  Returns (out0, out1, k, v).

Sharding: 8 cores = (batch b, token-half h): core c -> b=c//2, h=c%2.
Each core projects q/k/v for its 640 tokens (512 from x0, 128 from x1),
all-gathers k/v with its batch peer, runs attention for all 8 heads over
the full 2304-key axis in transposed (logitsT = [S, T]) layout, and applies
the output projections for its tokens.

Matmul operands are cast (on device) to fp16: fp32/fp32r moving operands
stream at ~2 cycles/row on the PE and their 4-byte weight loads (~200 ns,
no fast-weight-load) stall the array; fp16 runs at 1 cycle/row with FWL.
PSUM accumulation is fp32 throughout; k/v outputs are computed in fp32.

The attention mask is handled per 128(s) x 320(t) block, classified from the
actual input (union over cores): fully-masked blocks are skipped, fully-open
blocks need no mask op, mixed blocks get an additive mask loaded from DRAM.
"""
import numpy as np
from contextlib import ExitStack

import concourse.bacc as bacc
import concourse.tile as tile
import concourse.mybir as mybir
from concourse.alu_op_type import AluOpType
from concourse.bass_utils import run_bass_kernel_spmd

F32 = mybir.dt.float32
F16 = mybir.dt.float16
AF = mybir.ActivationFunctionType
PI = float(np.pi)
MAGIC = float(2**23)

B, T0, T1, SCC = 4, 1024, 256, 1024
T = T0 + T1                 # 1280
S = SCC + T                 # 2304
W0, W1 = 2048, 1024
NH, H = 8, 256
SCALE = float(H**-0.5)      # 1/16
EXP_SHIFT = -4.0            # softmax-invariant shift keeping exp in fp16 range

N_CORES = 8
T0C, T1C = T0 // 2, T1 // 2  # 512, 128 per core
TC = T0C + T1C               # 640
TCH = 320                    # attention t-chunk
N_TCH = TC // TCH            # 2
N_SCH = S // 128             # 18 s-chunks
BIG_NEG = -1.0e30
ZERO, ONES, MIXED = 0, 1, 2

_CACHE = {}


def _range_reduce_sin_cos(nc, pool, rad_ps, sin_dst, cos_dst, width):
    """From radians psum tile [128, width], write sin/cos into dst APs.

    sin arg reduced to [-pi, pi] via r - 2pi*round(r/2pi) (fp32 magic-round);
    cos arg reduced via r + pi/2 - 2pi*round(r/2pi + 0.25).
    """
    a = pool.tile([128, width], F32, tag="rr_a", name="rr_a")
    nc.vector.tensor_scalar(a[:], rad_ps[:], 1.0 / (2 * PI), MAGIC,
                            AluOpType.mult, AluOpType.add)
    b = pool.tile([128, width], F32, tag="rr_b", name="rr_b")
    nc.vector.tensor_scalar(b[:], a[:], MAGIC, -2 * PI,
                            AluOpType.subtract, AluOpType.mult)
    m = pool.tile([128, width], F32, tag="rr_c", name="rr_c")
    nc.vector.tensor_tensor(m[:], rad_ps[:], b[:], AluOpType.add)
    nc.scalar.activation(sin_dst, m[:], AF.Sin)

    a2 = pool.tile([128, width], F32, tag="rr_a", name="rr_a2")
    nc.vector.tensor_scalar(a2[:], rad_ps[:], 1.0 / (2 * PI), 0.25,
                            AluOpType.mult, AluOpType.add)
    b2 = pool.tile([128, width], F32, tag="rr_b", name="rr_b2")
    nc.vector.tensor_scalar(b2[:], a2[:], MAGIC, MAGIC,
                            AluOpType.add, AluOpType.subtract)
    c2 = pool.tile([128, width], F32, tag="rr_c", name="rr_c2")
    nc.vector.tensor_scalar(c2[:], b2[:], -2 * PI, PI / 2,
                            AluOpType.mult, AluOpType.add)
    m2 = pool.tile([128, width], F32, tag="rr_d", name="rr_d2")
    nc.vector.tensor_tensor(m2[:], rad_ps[:], c2[:], AluOpType.add)
    nc.scalar.activation(cos_dst, m2[:], AF.Sin)


def _rope_pair(nc, pool, out0_ap, out1_ap, p0, p1, ss, cc, width, tag):
    """out0 = p0*cc - p1*ss ; out1 = p1*cc + p0*ss (elementwise [128, width])."""
    t1 = pool.tile([128, width], F32, tag="rope_t1", name=f"rt1_{tag}")
    nc.vector.tensor_tensor(t1[:], p0, cc, AluOpType.mult)
    t2 = pool.tile([128, width], F32, tag="rope_t2", name=f"rt2_{tag}")
    nc.vector.tensor_tensor(t2[:], p1, ss, AluOpType.mult)
    nc.vector.tensor_tensor(out0_ap, t1[:], t2[:], AluOpType.subtract)
    t3 = pool.tile([128, width], F32, tag="rope_t1", name=f"rt3_{tag}")
    nc.vector.tensor_tensor(t3[:], p1, cc, AluOpType.mult)
    t4 = pool.tile([128, width], F32, tag="rope_t2", name=f"rt4_{tag}")
    nc.vector.tensor_tensor(t4[:], p0, ss, AluOpType.mult)
    nc.vector.tensor_tensor(out1_ap, t3[:], t4[:], AluOpType.add)


def _build(cls):
    """cls[tcn][sc] in {ZERO, ONES, MIXED} — mask block classification."""
    nc = bacc.Bacc(trn_type="TRN2", debug=False, num_devices=N_CORES)

    def din(name, shape):
        return nc.dram_tensor(name, shape, F32, kind="ExternalInput").ap()

    def dout(name, shape):
        return nc.dram_tensor(name, shape, F32, kind="ExternalOutput").ap()

    x0T_d = din("x0T", [W0, T0C])          # this core's x0 tokens, transposed
    x1T_d = din("x1T", [W1, T1C])
    wq0_d = din("wq0", [NH, W0, H])
    wkv0_d = din("wkv0", [2, W0, H])
    wq1_d = din("wq1", [NH, W1, H])
    wkv1_d = din("wkv1", [2, W1, H])
    wo0_d = din("wo0f", [NH * H, W0])      # flattened (n,h)-major
    wo1_d = din("wo1f", [NH * H, W1])
    ckT_d = din("ckT", [H, SCC])           # cache_k[b] transposed
    cv_d = din("cv", [SCC, H])             # cache_v[b]
    pos_d = din("pos", [1, TC])            # positions of this core's tokens
    invts_d = din("invts", [1, 128])       # 1/timescale
    ident_d = din("ident", [128, 128])     # identity for PE transpose
    maskT_d = din("maskT", [S, TC])        # additive mask, [s, t] layout

    out0_d = dout("out0", [T0C, W0])
    out1_d = dout("out1", [T1C, W1])
    kTn_d = dout("kTn", [H, TC])           # roped new k, [H, t]
    vn_d = dout("vn", [TC, H])             # new v, [t, H]

    with tile.TileContext(nc, num_cores=N_CORES) as tc, ExitStack() as ctx:
        # ---- whole-kernel pools ----
        pers = ctx.enter_context(tc.tile_pool(name="pers", bufs=1))
        qT_pool = ctx.enter_context(tc.tile_pool(name="qT", bufs=NH))
        encT_pool = ctx.enter_context(tc.tile_pool(name="encT", bufs=NH))
        dram = ctx.enter_context(tc.tile_pool(name="dram", bufs=1, space="DRAM"))

        ident = pers.tile([128, 128], F32)
        nc.sync.dma_start(ident[:], ident_d[:])
        ones_f = pers.tile([128, 128], F32)
        nc.vector.memset(ones_f[:], 1.0)
        ones_16 = pers.tile([128, 128], F16)
        nc.vector.tensor_copy(ones_16[:], ones_f[:])
        neg4 = pers.tile([128, 1], F32)
        nc.vector.memset(neg4[:], EXP_SHIFT)
        pos_sb = pers.tile([1, TC], F32)
        nc.sync.dma_start(pos_sb[:], pos_d[:])
        invts_sb = pers.tile([1, 128], F32)
        nc.sync.dma_start(invts_sb[:], invts_d[:])

        # rope tables ([i, t] layout for H-major rope; [t, i] for x1 path)
        sinT = pers.tile([128, TC], F32)
        cosT = pers.tile([128, TC], F32)
        sinTq = pers.tile([128, TC], F32)
        cosTq = pers.tile([128, TC], F32)
        sinF = pers.tile([128, 128], F32)
        cosF = pers.tile([128, 128], F32)
        sinFq = pers.tile([128, 128], F32)
        cosFq = pers.tile([128, 128], F32)
        with tc.tile_pool(name="ps_rad", bufs=2, space="PSUM") as ps_rad, \
             tc.tile_pool(name="rrtmp", bufs=2) as rrtmp:
            for half in range(2):
                sl = slice(half * TCH, (half + 1) * TCH)
                rad = ps_rad.tile([128, TCH], F32, tag="rad", name=f"radT{half}")
                nc.tensor.matmul(rad[:], invts_sb[:], pos_sb[:, sl],
                                 start=True, stop=True)
                _range_reduce_sin_cos(nc, rrtmp, rad, sinT[:, sl],
                                      cosT[:, sl], TCH)
            radF = ps_rad.tile([128, 128], F32, tag="rad", name="radF")
            nc.tensor.matmul(radF[:], pos_sb[:, T0C:TC], invts_sb[:],
                             start=True, stop=True)
            _range_reduce_sin_cos(nc, rrtmp, radF, sinF[:], cosF[:], 128)
        for src, dst in ((sinT, sinTq), (cosT, cosTq), (sinF, sinFq),
                         (cosF, cosFq)):
            nc.vector.tensor_scalar(dst[:], src[:], SCALE, None, AluOpType.mult)

        qts = [qT_pool.tile([128, 2 * TC], F16, tag="qT", name=f"qT_{n}")
               for n in range(NH)]

        # ---- projections ----
        with tc.tile_pool(name="xstage", bufs=1) as xstage, \
             tc.tile_pool(name="kvtmp", bufs=1) as kvtmp, \
             tc.tile_pool(name="tmp", bufs=2) as tmp_pool, \
             tc.tile_pool(name="wstage", bufs=3) as wstage, \
             tc.tile_pool(name="wbig", bufs=3) as wbig, \
             tc.tile_pool(name="wsmall", bufs=2) as wsmall, \
             tc.tile_pool(name="ps_qp", bufs=3, space="PSUM") as ps_qp, \
             tc.tile_pool(name="ps_vp", bufs=2, space="PSUM") as ps_vp, \
             tc.tile_pool(name="ps_tr", bufs=2, space="PSUM") as ps_tr:

            # x activations: DMA fp32 chunks, cast to one resident fp16 tile
            x0T_sb = xstage.tile([128, 16 * T0C], F16)   # [128, (dc t)]
            x0_src = x0T_d.rearrange("(c p) t -> p c t", p=128)
            for dc in range(16):
                stg = wstage.tile([128, T0C], F32, tag="wstage",
                                  name=f"x0stg_{dc}")
                nc.sync.dma_start(stg[:], x0_src[:, dc, :])
                nc.vector.tensor_copy(
                    x0T_sb[:, dc * T0C:(dc + 1) * T0C], stg[:])
            x1T_sb = xstage.tile([128, 8 * T1C], F16)
            x1_src = x1T_d.rearrange("(c p) t -> p c t", p=128)
            for dc2 in range(4):   # two 128-d chunks per stage tile
                stg = wstage.tile([128, 2 * T1C], F32, tag="x1stage",
                                  name=f"x1stg_{dc2}")
                nc.sync.dma_start(
                    stg[:].rearrange("p (c t) -> p c t", c=2),
                    x1_src[:, 2 * dc2: 2 * dc2 + 2, :])
                nc.vector.tensor_copy(
                    x1T_sb[:, 2 * dc2 * T1C:(2 * dc2 + 2) * T1C], stg[:])

            ktmp = kvtmp.tile([128, 2 * TC], F32)    # new k, [hc, t] free-major
            vtmp = kvtmp.tile([128, 5 * 256], F32)   # new v, 5 token-chunks

            def load_w16(w_ap, name, cast_engine="vector"):
                """[2048, 256] fp32 weight -> two [128, 8*256] fp16 tiles."""
                src = w_ap.rearrange("(c p) h -> p c h", p=128)
                tiles = []
                for hf in range(2):
                    stg = wstage.tile([128, 8 * H], F32, tag="wstage",
                                      name=f"{name}_s{hf}")
                    nc.sync.dma_start(
                        stg[:].rearrange("p (c h) -> p c h", c=8),
                        src[:, hf * 8:(hf + 1) * 8, :])
                    wt = wbig.tile([128, 8 * H], F16, tag="wbig",
                                   name=f"{name}_{hf}")
                    if cast_engine == "vector":
                        nc.vector.tensor_copy(wt[:], stg[:])
                    else:
                        nc.scalar.activation(wt[:], stg[:], AF.Copy)
                    tiles.append(wt)

                def sl(dc, off=0, width=H):
                    base = (dc % 8) * H + off
                    return tiles[dc // 8][:, base: base + width]
                return sl

            # x0 -> q (per head, [H,t] layout), via lhsT=wq0 chunks
            for n in range(NH):
                wq = load_w16(wq0_d[n], f"wq0_{n}", "scalar")
                qps = []
                for hc in range(2):
                    qp = ps_qp.tile([128, T0C], F32, tag="qp",
                                    name=f"qp_{n}_{hc}")
                    for dc in range(16):
                        nc.tensor.matmul(
                            qp[:],
                            wq(dc, hc * 128, 128),
                            x0T_sb[:, dc * T0C: (dc + 1) * T0C],
                            start=(dc == 0), stop=(dc == 15))
                    qps.append(qp)
                _rope_pair(nc, tmp_pool, qts[n][:, 0:T0C],
                           qts[n][:, TC:TC + T0C], qps[0][:], qps[1][:],
                           sinTq[:, 0:T0C], cosTq[:, 0:T0C], T0C, f"q{n}")

            # x0 -> k ([H,t] layout)
            wk = load_w16(wkv0_d[0], "wk0", "scalar")
            kps = []
            for hc in range(2):
                kp = ps_qp.tile([128, T0C], F32, tag="qp", name=f"kp_{hc}")
                for dc in range(16):
                    nc.tensor.matmul(
                        kp[:],
                        wk(dc, hc * 128, 128),
                        x0T_sb[:, dc * T0C: (dc + 1) * T0C],
                        start=(dc == 0), stop=(dc == 15))
                kps.append(kp)
            _rope_pair(nc, tmp_pool, ktmp[:, 0:T0C], ktmp[:, TC:TC + T0C],
                       kps[0][:], kps[1][:], sinT[:, 0:T0C], cosT[:, 0:T0C],
                       T0C, "k")

            # x0 -> v ([t, H] layout)
            wv = load_w16(wkv0_d[1], "wv0", "scalar")
            for tcn in range(4):
                vp = ps_vp.tile([128, H], F32, tag="vp", name=f"vp_{tcn}")
                for dc in range(16):
                    nc.tensor.matmul(
                        vp[:],
                        x0T_sb[:, dc * T0C + tcn * 128: dc * T0C + tcn * 128 + 128],
                        wv(dc),
                        start=(dc == 0), stop=(dc == 15))
                nc.vector.tensor_copy(vtmp[:, tcn * H: (tcn + 1) * H], vp[:])

            # x1 -> q/k in [t, H] then PE-transpose into [H, t]
            def x1_proj(w_d, name):
                src = w_d.rearrange("(c p) h -> p c h", p=128)
                stg = wstage.tile([128, 8 * H], F32, tag="wstage",
                                  name=f"ws_{name}")
                nc.sync.dma_start(
                    stg[:].rearrange("p (c h) -> p c h", c=8), src)
                ww = wsmall.tile([128, 8 * H], F16, tag="wsmall",
                                 name=f"w_{name}")
                nc.scalar.activation(ww[:], stg[:], AF.Copy)
                pp = ps_vp.tile([128, H], F32, tag="vp", name=f"pp_{name}")
                for dc in range(8):
                    nc.tensor.matmul(
                        pp[:],
                        x1T_sb[:, dc * T1C: (dc + 1) * T1C],
                        ww[:, dc * H: (dc + 1) * H],
                        start=(dc == 0), stop=(dc == 7))
                return pp

            def rope_F(pp, scaled, name):
                ss, cc = (sinFq, cosFq) if scaled else (sinF, cosF)
                ro = tmp_pool.tile([128, H], F32, tag="ropeF", name=f"ro_{name}")
                _rope_pair(nc, tmp_pool, ro[:, 0:128], ro[:, 128:256],
                           pp[:, 0:128], pp[:, 128:256], ss[:], cc[:], 128,
                           name)
                return ro

            def transpose_to(ro, out_tile, base_off, name):
                for hc in range(2):
                    tp = ps_tr.tile([128, 128], F32, tag="tr",
                                    name=f"tp_{name}_{hc}")
                    nc.tensor.transpose(tp[:], ro[:, hc * 128:(hc + 1) * 128],
                                        ident[:])
                    nc.vector.tensor_copy(
                        out_tile[:, base_off + hc * TC:
                                 base_off + hc * TC + T1C], tp[:])

            for n in range(NH):
                pp = x1_proj(wq1_d[n], f"q1_{n}")
                ro = rope_F(pp, True, f"q1_{n}")
                transpose_to(ro, qts[n], T0C, f"q1_{n}")

            pp = x1_proj(wkv1_d[0], "k1")
            ro = rope_F(pp, False, "k1")
            # k1 transpose goes into fp32 ktmp
            for hc in range(2):
                tp = ps_tr.tile([128, 128], F32, tag="tr", name=f"tp_k1_{hc}")
                nc.tensor.transpose(tp[:], ro[:, hc * 128:(hc + 1) * 128],
                                    ident[:])
                nc.vector.tensor_copy(
                    ktmp[:, T0C + hc * TC: T0C + hc * TC + T1C], tp[:])

            pp = x1_proj(wkv1_d[1], "v1")
            nc.vector.tensor_copy(vtmp[:, 4 * H: 5 * H], pp[:])

            # ---- fp32 outputs for new k/v ----
            for hc in range(2):
                nc.sync.dma_start(kTn_d[hc * 128:(hc + 1) * 128, :],
                                  ktmp[:, hc * TC:(hc + 1) * TC])
            nc.sync.dma_start(vn_d.rearrange("(c p) h -> p c h", p=128),
                              vtmp[:].rearrange("p (c h) -> p c h", c=5))

            # ---- fp16 copies + allgather ----
            kt16 = kvtmp.tile([128, 2 * TC], F16)
            nc.vector.tensor_copy(kt16[:], ktmp[:])
            vt16 = kvtmp.tile([128, 5 * 256], F16)
            nc.vector.tensor_copy(vt16[:], vtmp[:])

            KBLOB = 2 * 128 * TC        # fp16 elems
            VBLOB = 5 * 128 * H
            bnc_in = dram.tile([KBLOB + VBLOB], F16)
            bnc_out = dram.tile([2, KBLOB + VBLOB], F16)
            nc.sync.dma_start(
                bnc_in[0:KBLOB].rearrange("(hc p t) -> p hc t", hc=2, p=128),
                kt16[:].rearrange("p (hc t) -> p hc t", hc=2))
            nc.sync.dma_start(
                bnc_in[KBLOB:KBLOB + VBLOB].rearrange("(c p h) -> p c h",
                                                      c=5, p=128),
                vt16[:].rearrange("p (c h) -> p c h", c=5))
            nc.gpsimd.collective_compute(
                "AllGather", mybir.AluOpType.bypass,
                replica_groups=[[0, 1], [2, 3], [4, 5], [6, 7]],
                ins=[bnc_in[:]], outs=[bnc_out[:]])

        # ---- assemble kT [128, (hc s)] and v [128, (sc h)] in fp16 ----
        kv_pool = ctx.enter_context(tc.tile_pool(name="kv", bufs=1))
        kT_sb = kv_pool.tile([128, 2 * S], F16)
        v_sb = kv_pool.tile([128, N_SCH * H], F16)
        with tc.tile_pool(name="cstage", bufs=2) as cstage:
            for hc in range(2):
                stg = cstage.tile([128, SCC], F32, tag="cstage",
                                  name=f"ck_{hc}")
                nc.sync.dma_start(stg[:], ckT_d[hc * 128:(hc + 1) * 128, :])
                nc.vector.tensor_copy(kT_sb[:, hc * S: hc * S + SCC], stg[:])
            for cvh in range(2):
                stg = cstage.tile([128, SCC], F32, tag="cstage",
                                  name=f"cv_{cvh}")
                nc.sync.dma_start(
                    stg[:].rearrange("p (c h) -> p c h", c=4),
                    cv_d.rearrange("(c p) h -> p c h", p=128)[:, cvh * 4:
                                                             cvh * 4 + 4, :])
                nc.vector.tensor_copy(
                    v_sb[:, cvh * 4 * H: (cvh * 4 + 4) * H], stg[:])
        KBLOB = 2 * 128 * TC
        VBLOB = 5 * 128 * H
        for hh in range(2):
            kblob = bnc_out[hh, 0:KBLOB].rearrange(
                "(hc p t) -> p hc t", hc=2, p=128)
            for hc in range(2):
                nc.sync.dma_start(
                    kT_sb[:, hc * S + SCC + hh * T0C:
                          hc * S + SCC + hh * T0C + T0C],
                    kblob[:, hc, 0:T0C])
                nc.sync.dma_start(
                    kT_sb[:, hc * S + SCC + T0 + hh * T1C:
                          hc * S + SCC + T0 + hh * T1C + T1C],
                    kblob[:, hc, T0C:TC])
            vblob = bnc_out[hh, KBLOB:KBLOB + VBLOB].rearrange(
                "(c p h) -> p c h", c=5, p=128)
            nc.sync.dma_start(
                v_sb[:, (8 + hh * 4) * H: (8 + hh * 4) * H + 4 * H]
                .rearrange("p (c h) -> p c h", c=4),
                vblob[:, 0:4, :])
            nc.sync.dma_start(
                v_sb[:, (16 + hh) * H: (16 + hh) * H + H], vblob[:, 4, :])

        encs = [encT_pool.tile([128, 2 * TC], F16, tag="encT",
                               name=f"encT_{n}") for n in range(NH)]

        # ---- attention (logitsT layout, shifted exp, no max-subtraction) ----
        with tc.tile_pool(name="maskp", bufs=12) as maskp, \
             tc.tile_pool(name="ep", bufs=6) as ep, \
             tc.tile_pool(name="recipp", bufs=2) as recipp, \
             tc.tile_pool(name="ps_lg", bufs=2, space="PSUM") as ps_lg, \
             tc.tile_pool(name="ps_enc", bufs=4, space="PSUM") as ps_enc, \
             tc.tile_pool(name="ps_den", bufs=2, space="PSUM") as ps_den:
            for tcn in range(N_TCH):
                tsl = slice(tcn * TCH, (tcn + 1) * TCH)
                active = [sc for sc in range(N_SCH) if cls[tcn][sc] != ZERO]
                mtiles = {}
                for sc in active:
                    if cls[tcn][sc] == MIXED:
                        mk = maskp.tile([128, TCH], F32, tag="mask",
                                        name=f"mk_{tcn}_{sc}")
                        nc.sync.dma_start(
                            mk[:], maskT_d[sc * 128:(sc + 1) * 128, tsl])
                        mtiles[sc] = mk
                for n in range(NH):
                    qT = qts[n]
                    enc0 = ps_enc.tile([128, TCH], F32, tag="enc",
                                       name=f"enc0_{tcn}_{n}")
                    enc1 = ps_enc.tile([128, TCH], F32, tag="enc",
                                       name=f"enc1_{tcn}_{n}")
                    den = ps_den.tile([128, TCH], F32, tag="den",
                                      name=f"den_{tcn}_{n}")
                    for i, sc in enumerate(active):
                        lg = ps_lg.tile([128, TCH], F32, tag="lg",
                                        name=f"lg_{tcn}_{n}_{sc}")
                        for hc in range(2):
                            nc.tensor.matmul(
                                lg[:],
                                kT_sb[:, hc * S + sc * 128:
                                      hc * S + sc * 128 + 128],
                                qT[:, hc * TC + tcn * TCH:
                                   hc * TC + tcn * TCH + TCH],
                                start=(hc == 0), stop=(hc == 1))
                        if sc in mtiles:
                            nc.vector.tensor_tensor(lg[:], lg[:],
                                                    mtiles[sc][:],
                                                    AluOpType.add)
                        eT = ep.tile([128, TCH], F16, tag="eT",
                                     name=f"eT_{tcn}_{n}_{sc}")
                        nc.scalar.activation(eT[:], lg[:], AF.Exp,
                                             bias=neg4[:])
                        first, last = (i == 0), (i == len(active) - 1)
                        for hc, enc in ((0, enc0), (1, enc1)):
                            nc.tensor.matmul(
                                enc[:],
                                v_sb[:, sc * H + hc * 128:
                                     sc * H + hc * 128 + 128],
                                eT[:], start=first, stop=last)
                        nc.tensor.matmul(den[:], ones_16[:], eT[:],
                                         start=first, stop=last)
                    recip = recipp.tile([128, TCH], F32, tag="recip",
                                        name=f"rc_{tcn}_{n}")
                    nc.vector.reciprocal(recip[:], den[:])
                    for hc, enc in ((0, enc0), (1, enc1)):
                        nc.vector.tensor_tensor(
                            encs[n][:, hc * TC + tcn * TCH:
                                    hc * TC + tcn * TCH + TCH],
                            enc[:], recip[:], AluOpType.mult)

        # ---- output projections ----
        with tc.tile_pool(name="wostage", bufs=5) as wostage, \
             tc.tile_pool(name="wo", bufs=5) as wop, \
             tc.tile_pool(name="osb", bufs=4) as osb, \
             tc.tile_pool(name="ps_out", bufs=6, space="PSUM") as ps_out:

            def load_wo(w_d, kc, d, name):
                stg = wostage.tile([128, 512], F32, tag="wos",
                                   name=f"s_{name}")
                nc.gpsimd.dma_start(
                    stg[:], w_d[kc * 128:(kc + 1) * 128, d * 512:(d + 1) * 512])
                rhs = wop.tile([128, 512], F16, tag="wo", name=name)
                nc.vector.tensor_copy(rhs[:], stg[:])
                return rhs

            for d in range(4):
                outp = [ps_out.tile([128, 512], F32, tag="po",
                                    name=f"po0_{d}_{t4}") for t4 in range(4)]
                for kc in range(16):
                    n, hc = kc // 2, kc % 2
                    rhs = load_wo(wo0_d, kc, d, f"wo0_{d}_{kc}")
                    for t4 in range(4):
                        nc.tensor.matmul(
                            outp[t4][:],
                            encs[n][:, hc * TC + t4 * 128:
                                    hc * TC + t4 * 128 + 128],
                            rhs[:], start=(kc == 0), stop=(kc == 15))
                for t4 in range(4):
                    ot = osb.tile([128, 512], F32, tag="ot", name=f"ot0_{d}_{t4}")
                    nc.vector.tensor_copy(ot[:], outp[t4][:])
                    nc.sync.dma_start(
                        out0_d[t4 * 128:(t4 + 1) * 128, d * 512:(d + 1) * 512],
                        ot[:])
            for d in range(2):
                op1 = ps_out.tile([128, 512], F32, tag="po", name=f"po1_{d}")
                for kc in range(16):
                    n, hc = kc // 2, kc % 2
                    rhs = load_wo(wo1_d, kc, d, f"wo1_{d}_{kc}")
                    nc.tensor.matmul(
                        op1[:],
                        encs[n][:, hc * TC + T0C: hc * TC + TC],
                        rhs[:], start=(kc == 0), stop=(kc == 15))
                ot = osb.tile([128, 512], F32, tag="ot", name=f"ot1_{d}")
                nc.vector.tensor_copy(ot[:], op1[:])
                nc.sync.dma_start(
                    out1_d[:, d * 512:(d + 1) * 512], ot[:])

    nc.compile()
    return nc


def _prep_inputs(inputs):
    """Host-side staging: slice/transpose per core; classify mask blocks."""
    x0 = np.ascontiguousarray(inputs["x0"], dtype=np.float32)
    x1 = np.ascontiguousarray(inputs["x1"], dtype=np.float32)
    wq0 = np.ascontiguousarray(inputs["wq0"], dtype=np.float32)
    wkv0 = np.ascontiguousarray(np.asarray(inputs["wkv0"], dtype=np.float32)[:, 0])
    wo0 = np.ascontiguousarray(inputs["wo0"], dtype=np.float32)
    wq1 = np.ascontiguousarray(inputs["wq1"], dtype=np.float32)
    wkv1 = np.ascontiguousarray(np.asarray(inputs["wkv1"], dtype=np.float32)[:, 0])
    wo1 = np.ascontiguousarray(inputs["wo1"], dtype=np.float32)
    cache_k = np.asarray(inputs["cache_k"], dtype=np.float32)[:, :, 0]
    cache_v = np.asarray(inputs["cache_v"], dtype=np.float32)[:, :, 0]
    positions = np.asarray(inputs["positions"], dtype=np.float32)
    mask = np.asarray(inputs["attn_mask"])[:, 0]          # [B, T, S] bool

    wo0f = np.ascontiguousarray(wo0.reshape(NH * H, W0))
    wo1f = np.ascontiguousarray(wo1.reshape(NH * H, W1))
    half = H // 2
    invts = (10000.0 ** (-(2.0 / H) * np.arange(half, dtype=np.float32))
             ).astype(np.float32).reshape(1, half)
    ident = np.eye(128, dtype=np.float32)

    in_maps = []
    maskTs = []
    for c in range(N_CORES):
        b, h = divmod(c, 2)
        sl0 = slice(h * T0C, (h + 1) * T0C)
        sl1 = slice(T0 + h * T1C, T0 + (h + 1) * T1C)
        x0T = np.ascontiguousarray(x0[b, sl0].T)          # [W0, 512]
        x1T = np.ascontiguousarray(x1[b, h * T1C:(h + 1) * T1C].T)
        pos = np.concatenate([positions[b, sl0], positions[b, sl1]]
                             ).reshape(1, TC).astype(np.float32)
        m_rows = np.concatenate([mask[b, sl0], mask[b, sl1]], axis=0)
        maskTs.append(m_rows.T)                           # [S, 640] bool
        maskT = np.where(m_rows.T, np.float32(0.0), np.float32(BIG_NEG))
        ckT = np.ascontiguousarray(cache_k[b].T)          # [H, 1024]
        in_maps.append({
            "x0T": x0T, "x1T": x1T,
            "wq0": wq0, "wkv0": wkv0, "wq1": wq1, "wkv1": wkv1,
            "wo0f": wo0f, "wo1f": wo1f,
            "ckT": ckT, "cv": np.ascontiguousarray(cache_v[b]),
            "pos": pos, "invts": invts, "ident": ident,
            "maskT": np.ascontiguousarray(maskT),
        })

    allm = np.stack(maskTs)                               # [8, S, 640] bool
    cls = []
    for tcn in range(N_TCH):
        row = []
        for sc in range(N_SCH):
            blk = allm[:, sc * 128:(sc + 1) * 128,
                       tcn * TCH:(tcn + 1) * TCH]
            if blk.all():
                row.append(ONES)
            elif not blk.any():
                row.append(ZERO)
            else:
                row.append(MIXED)
        cls.append(tuple(row))
    return in_maps, tuple(cls)


def kernel(**inputs):
    in_maps, cls = _prep_inputs(inputs)
    if cls not in _CACHE:
        _CACHE[cls] = _build(cls)
    nc = _CACHE[cls]
    res = run_bass_kernel_spmd(nc, in_maps, core_ids=list(range(N_CORES)))

    out0 = np.empty((B, T0, W0), dtype=np.float32)
    out1 = np.empty((B, T1, W1), dtype=np.float32)
    k = np.empty((B, S, 1, H), dtype=np.float32)
    v = np.empty((B, S, 1, H), dtype=np.float32)
    k[:, :SCC] = np.asarray(inputs["cache_k"], dtype=np.float32)
    v[:, :SCC] = np.asarray(inputs["cache_v"], dtype=np.float32)
    for c in range(N_CORES):
        b, h = divmod(c, 2)
        r = res.results[c]
        out0[b, h * T0C:(h + 1) * T0C] = r["out0"]
        out1[b, h * T1C:(h + 1) * T1C] = r["out1"]
        kTn = r["kTn"]                                    # [H, 640]
        vn = r["vn"]                                      # [640, H]
        k[b, SCC + h * T0C: SCC + (h + 1) * T0C, 0] = kTn[:, :T0C].T
        k[b, SCC + T0 + h * T1C: SCC + T0 + (h + 1) * T1C, 0] = kTn[:, T0C:].T
        v[b, SCC + h * T0C: SCC + (h + 1) * T0C, 0] = vn[:T0C]
        v[b, SCC + T0 + h * T1C: SCC + T0 + (h + 1) * T1C, 0] = vn[T0C:]
    return out0, out1, k, v
